# revision 19
# baseline (speedup 1.0000x reference)
"""TRN2 Bass kernel for the discrete dense Koopman operator rollout.

    z_{t+1} = z_t @ K ;  output[b, t, d] = (z0 @ K^{t+1})[b, d],  t = 0..255

Strategy (batch sharding, SPMD across 8 NeuronCores):
  - core m owns batch rows 32m .. 32m+31 and computes ALL 256 time steps
    for them (the scan is embarrassingly parallel over batch).
  - on-device setup per core (identical instruction stream):
      * squaring ladder K -> K^2 -> K^4 -> K^8 -> K^16 (4 squarings;
        host supplies K^T to seed the first squaring; intermediate
        transposes via PE transpose-mode)
      * seed rows z_1..z_16 by state doubling:
          z_1 = z_0 K ; z_2 = z_1 K ; [z_3 z_4] = [z_1 z_2] K^2 ;
          [z_5..z_8] = [z_1..z_4] K^4 ; [z_9..z_16] = [z_1..z_8] K^8
      * 15 rounds: block_{r+1} = block_r @ K^16, where a block is 16
        time steps x 32 batch = a [512, 512] feature-major tile group ->
        each round is 16 accumulating [128x128]@[128,512] matmuls (the
        moving stream fully hides the f32r weight load, and casts/DMAs
        have ~3 us of slack inside a 3.6 us round -> no just-in-time
        stalls on the PE).
  - matmuls run as float32r (e8m11, RNE; 1 cycle/row).  Inputs are
    pre-rounded on the host (bit-exact same RNE); PSUM accumulation is
    exact fp32; the rounded state is DMA'd out directly as fp32 output.
  - state kept feature-major (ZT = z^T, [D, b]) so K blocks are the
    stationary operand and no per-step transposes are needed.
  - output DRAM layout is feature-major [D, T, BC] so each output DMA
    writes 2 KiB contiguous per partition (time-major would shatter
    DMAs into 128 B segments at ~4x the cost); the host transposes.
  - startup: K^T/K chunk i ride the two HWDGE queues in lockstep and the
    first squaring consumes them in arrival order; z0 queues behind the
    K^T loads.  A single long accumulation group of identity matmuls
    warms the PE (HAM un-throttle) while the first K chunks' DMA
    completion semaphores are still in flight.
  - squarings after the first run output-chunk-outer, so their casts
    stagger and every next phase's first matmul finds its operand ready
    (no inter-phase bubbles); seed groups slot between ladder phases to
    cover the two cast bursts that do bunch (K^2's c-outer tail).

kernel() takes FULL inputs and returns the FULL output.
"""

import os
import sys
import numpy as np

import concourse.bass as bass
import concourse.tile as tile
import concourse.mybir as mybir
from concourse.bass import ts, ds
from concourse import bass_utils, bacc
from concourse.masks import make_identity

dt = mybir.dt
F32, F32R = dt.float32, dt.float32r

B, D, T_STEPS = 256, 512, 256
NCORES = 8
BC = B // NCORES                # 32 batch rows per core
SB = 16                         # time steps per block
NB = T_STEPS // SB              # 16 blocks (1 seed + 15 rounds)
NR = SB * BC                    # 512 state columns per block
DP = D // 128                   # 4 partition chunks of the feature dim
N_WARMUP = 30                   # identity matmuls before first real MM


def wavefront():
    """(i, j) pairs in anti-diagonal order; i ascending within a group j."""
    for w in range(2 * DP - 1):
        for i in range(max(0, w - DP + 1), min(DP, w + 1)):
            yield i, w - i


def build_nc():
    nc = bacc.Bacc("TRN2", target_bir_lowering=False, debug=False,
                   num_devices=NCORES)
    # all tensor inputs pre-rounded to f32r (e8m11, RNE) on the host
    zt_d = nc.dram_tensor("zt_in", [D, BC], F32R, kind="ExternalInput").ap()
    k_d = nc.dram_tensor("k_in", [D, D], F32R, kind="ExternalInput").ap()
    kt_d = nc.dram_tensor("kt_in", [D, D], F32R, kind="ExternalInput").ap()
    out_d = nc.dram_tensor("out", [D, T_STEPS, BC], F32,
                           kind="ExternalOutput").ap()

    with tile.TileContext(nc) as tc:
        with tc.tile_pool(name="const", bufs=1) as cp, \
             tc.tile_pool(name="pow", bufs=1) as powp, \
             tc.tile_pool(name="state", bufs=3) as stp, \
             tc.tile_pool(name="psum", bufs=2, space="PSUM") as pp:

            Kr, KTr = [], []
            for i in range(DP):
                ktr = cp.tile([128, D], F32R, name=f"KTr{i}", tag=f"KTr{i}")
                KTr.append(ktr)
                kr = cp.tile([128, D], F32R, name=f"Kr{i}", tag=f"Kr{i}")
                Kr.append(kr)

            def load_k_chunk(i):
                nc.sync.dma_start(KTr[i][:], kt_d[ts(i, 128), :])
                nc.scalar.dma_start(Kr[i][:], k_d[ts(i, 128), :])

            load_k_chunk(0)

            identf = cp.tile([128, 128], F32, name="identf", tag="identf")
            make_identity(nc, identf[:])
            ident = cp.tile([128, 128], F32R, name="ident", tag="ident")
            nc.vector.tensor_copy(ident[:], identf[:])

            # PE warm-up while the K chunk-0 DMA completion semaphores are
            # still in flight.  One long accumulation group: back-to-back
            # at LDW cadence (independent single-MM groups serialize on
            # fill/drain and leave the array half idle -> HAM stays
            # throttled).
            with nc.named_scope("warmup"):
                pw = pp.tile([128, 128], F32, name="wu", tag="p0")
                for w in range(N_WARMUP):
                    nc.tensor.matmul(pw[:], ident[:], ident[:],
                                     start=(w == 0),
                                     stop=(w == N_WARMUP - 1))

            def square(ATr, Ar, name, c_outer=False):
                """(A @ A) as f32r tiles. ATr: lhsT (A^T); Ar: rhs (A)."""
                pss = [pp.tile([128, D], F32, name=f"sq_{name}_{ib}",
                               tag=f"p{ib}") for ib in range(DP)]
                order = ([(c, ib) for c in range(DP) for ib in range(DP)]
                         if c_outer else
                         [(c, ib) for ib in range(DP) for c in range(DP)])
                for c, ib in order:
                    if c_outer and ib == 0 and c + 1 < DP:
                        load_k_chunk(c + 1)
                    if c_outer and ib == 0 and c > 0:
                        # trivial identity transpose: breaks the coalesced
                        # PE semaphore run so this c-group waits only on
                        # its own K chunks, not on all of them.
                        dum = pp.tile([128, 128], F32R, name=f"dum{name}{c}",
                                      tag=f"p{c}")
                        nc.tensor.transpose(dum[:], ident[:], ident[:])
                    nc.tensor.matmul(pss[ib][:],
                                     ATr[c][:, ts(ib, 128)],
                                     Ar[c][:],
                                     start=(c == 0), stop=(c == DP - 1))
                out_tiles = []
                for ib in range(DP):
                    r = powp.tile([128, D], F32R, name=f"{name}r_{ib}",
                                  tag=f"{name}r_{ib}")
                    if ib % 2 == 0:
                        nc.vector.tensor_copy(r[:], pss[ib][:])
                    else:
                        nc.scalar.copy(r[:], pss[ib][:])
                    out_tiles.append(r)
                return out_tiles

            def transpose_r(Ar, name):
                """PE-transpose the f32r tiles (values exact)."""
                outs = [powp.tile([128, D], F32R, name=f"{name}_{c}",
                                  tag=f"{name}_{c}") for c in range(DP)]
                for c in range(DP):
                    for i in range(DP):
                        ps = pp.tile([128, 128], F32R, name=f"t_{name}_{i}{c}",
                                     tag=f"p{c}")
                        nc.tensor.transpose(ps[:], Ar[i][:, ts(c, 128)],
                                            ident[:])
                        if (i + c) % 2 == 0:
                            nc.vector.tensor_copy(outs[c][:, ts(i, 128)],
                                                  ps[:])
                        else:
                            nc.scalar.copy(outs[c][:, ts(i, 128)], ps[:])
                return outs

            # seed block: X[ib][:, s*BC:(s+1)*BC] = (z_{s+1})^T chunk,
            # s = 0..15
            X = [stp.tile([128, NR], F32R, name=f"x0_{ib}",
                          tag=f"x{ib}") for ib in range(DP)]

            def seed_group(lhsT, rhs_of, n, dst_lo, name):
                pss = [pp.tile([128, n], F32, name=f"{name}_{jb}",
                               tag=f"p{jb}") for jb in range(DP)]
                done = [0] * DP
                for i, jb in wavefront():
                    nc.tensor.matmul(pss[jb][:],
                                     lhsT[i][:, ts(jb, 128)],
                                     rhs_of(i),
                                     start=(i == 0), stop=(i == DP - 1))
                    done[jb] += 1
                    if done[jb] == DP:
                        dst = X[jb][:, dst_lo * BC:dst_lo * BC + n]
                        if jb % 2 == 0:
                            nc.vector.tensor_copy(dst, pss[jb][:])
                        else:
                            nc.scalar.copy(dst, pss[jb][:])

            with nc.named_scope("ladder"):
                K2r = square(KTr, Kr, "K2", c_outer=True)
            z0t = [cp.tile([128, BC], F32R, name=f"z0t{i}", tag=f"z0t{i}")
                   for i in range(DP)]
            for i in range(DP):
                nc.sync.dma_start(z0t[i][:], zt_d[ts(i, 128), :])

            with nc.named_scope("seed"):
                seed_group(Kr, lambda i: z0t[i][:], BC, 0, "s0")
            with nc.named_scope("ladder"):
                K2Tr = transpose_r(K2r, "K2T")
            with nc.named_scope("seed"):
                seed_group(Kr, lambda i: X[i][:, 0:BC], BC, 1, "s1")
            with nc.named_scope("ladder"):
                K4r = square(K2Tr, K2r, "K4")
            with nc.named_scope("seed"):
                seed_group(K2r, lambda i: X[i][:, 0:2 * BC], 2 * BC, 2, "s2")
            with nc.named_scope("ladder"):
                K4Tr = transpose_r(K4r, "K4T")
            with nc.named_scope("seed"):
                seed_group(K4r, lambda i: X[i][:, 0:4 * BC], 4 * BC, 4, "s3")
            with nc.named_scope("ladder"):
                K8r = square(K4Tr, K4r, "K8")
            with nc.named_scope("seed"):
                seed_group(K8r, lambda i: X[i][:, 0:8 * BC], 8 * BC, 8, "s4")
            with nc.named_scope("ladder"):
                K8Tr = transpose_r(K8r, "K8T")
                K16r = square(K8Tr, K8r, "K16")
            # seed block -> output rows 0..15
            for ib in range(DP):
                dma_eng = nc.sync if ib % 2 == 0 else nc.scalar
                dma_eng.dma_start(
                    out_d[ts(ib, 128), ds(0, SB), :],
                    X[ib][:].bitcast(F32))

            # ---------------- phase B: K^16 rounds ----------------
            with nc.named_scope("rounds"):
                for r in range(1, NB):
                    pss = [pp.tile([128, NR], F32, name=f"rd{r}_{jb}",
                                   tag=f"p{jb}") for jb in range(DP)]
                    done = [0] * DP
                    nxt = [None] * DP
                    for i, jb in wavefront():
                        nc.tensor.matmul(pss[jb][:],
                                         K16r[i][:, ts(jb, 128)],
                                         X[i][:],
                                         start=(i == 0), stop=(i == DP - 1))
                        done[jb] += 1
                        if done[jb] == DP:
                            o = stp.tile([128, NR], F32R,
                                         name=f"x{r}_{jb}", tag=f"x{jb}")
                            nc.vector.tensor_copy(o[:], pss[jb][:])
                            dma_eng = nc.sync if jb % 2 == 0 else nc.scalar
                            dma_eng.dma_start(
                                out_d[ts(jb, 128), ds(SB * r, SB), :],
                                o[:].bitcast(F32))
                            nxt[jb] = o
                    X = nxt

    nc.compile()
    return nc


def _round_f32r(x):
    """RNE round fp32 -> f32r (e8m11): bit-exact match of the HW/DVE cast."""
    b = x.view(np.uint32).astype(np.uint64)
    keep = b >> 12
    rem = b & 0xFFF
    rup = (rem > 0x800) | ((rem == 0x800) & ((keep & 1) == 1))
    return ((keep + rup) << 12).astype(np.uint32).view(np.float32).copy()


_CACHE = {}


def kernel(z0, K, T):
    z0 = np.asarray(z0, dtype=np.float32)
    K = np.asarray(K, dtype=np.float32)
    T = int(T)
    assert z0.shape == (B, D) and K.shape == (D, D) and T == T_STEPS

    if "nc" not in _CACHE:
        _CACHE["nc"] = build_nc()
    nc = _CACHE["nc"]

    Kr = _round_f32r(np.ascontiguousarray(K))
    zt = _round_f32r(np.ascontiguousarray(z0.T))      # [D, B]
    kt = np.ascontiguousarray(Kr.T)                   # [D, D] (round then T)
    in_maps = []
    for m in range(NCORES):
        in_maps.append({
            "zt_in": np.ascontiguousarray(zt[:, m * BC:(m + 1) * BC]),
            "k_in": Kr, "kt_in": kt})

    trace = bool(os.environ.get("KOOPMAN_TRACE"))
    if trace:
        _install_ntff_hook()
    res = bass_utils.run_bass_kernel_spmd(
        nc, in_maps, core_ids=list(range(NCORES)),
        trace=trace, trace_cores=[0] if trace else None)
    if trace:
        _CACHE["last_result"] = res

    # assemble: per-core out [D, T, BC] -> full [B, T, D]
    full = np.empty((B, T_STEPS, D), dtype=np.float32)
    for m in range(NCORES):
        blk = res.results[m]["out"]               # [D, T, BC]
        full[m * BC:(m + 1) * BC, :, :] = blk.transpose(2, 1, 0)
    return full


def _install_ntff_hook():
    """Dev-only: register the axon NTFF profiling hook (absent from this
    image's antenv) so trace=True works."""
    import types
    if "antenv.axon_hooks" in sys.modules:
        return
    try:
        from trn_agent_boot.trn_boot import _ntff_profile_via_ctypes
        hook = _ntff_profile_via_ctypes("/opt/axon/libaxon_pjrt.so")
    except Exception:
        return
    mod = types.ModuleType("antenv.axon_hooks")
    mod.get_axon_ntff_profile_hook = lambda: hook
    mod.set_axon_ntff_profile_hook = lambda h: None
    sys.modules["antenv.axon_hooks"] = mod


# revision 27
# speedup vs baseline: 1.0236x; 1.0236x over previous
"""TRN2 Bass kernel for the discrete dense Koopman operator rollout.

    z_{t+1} = z_t @ K ;  output[b, t, d] = (z0 @ K^{t+1})[b, d],  t = 0..255

Strategy (batch sharding, SPMD across 8 NeuronCores):
  - core m owns batch rows 32m .. 32m+31 and computes ALL 256 time steps
    for them (the scan is embarrassingly parallel over batch).
  - on-device setup per core (identical instruction stream):
      * squaring ladder K -> K^2 -> K^4 -> K^8 -> K^16 (4 squarings;
        host supplies K^T to seed the first squaring; intermediate
        transposes via PE transpose-mode)
      * seed rows z_1..z_16 by state doubling:
          z_1 = z_0 K ; z_2 = z_1 K ; [z_3 z_4] = [z_1 z_2] K^2 ;
          [z_5..z_8] = [z_1..z_4] K^4 ; [z_9..z_16] = [z_1..z_8] K^8
      * 15 rounds: block_{r+1} = block_r @ K^16, where a block is 16
        time steps x 32 batch = a [512, 512] feature-major tile group ->
        each round is 16 accumulating [128x128]@[128,512] matmuls (the
        moving stream fully hides the f32r weight load, and casts/DMAs
        have ~3 us of slack inside a 3.6 us round -> no just-in-time
        stalls on the PE).
  - matmuls run as float32r (e8m11, RNE; 1 cycle/row).  Inputs are
    pre-rounded on the host (bit-exact same RNE); PSUM accumulation is
    exact fp32; the rounded state is DMA'd out directly as fp32 output.
  - state kept feature-major (ZT = z^T, [D, b]) so K blocks are the
    stationary operand and no per-step transposes are needed.
  - output DRAM layout is feature-major [D, T, BC] so each output DMA
    writes 2 KiB contiguous per partition (time-major would shatter
    DMAs into 128 B segments at ~4x the cost); the host transposes.
  - startup: K^T/K chunk i ride the two HWDGE queues in lockstep and the
    first squaring consumes them in arrival order; z0 queues behind the
    K^T loads.  A single long accumulation group of identity matmuls
    warms the PE (HAM un-throttle) while the first K chunks' DMA
    completion semaphores are still in flight.
  - squarings after the first run output-chunk-outer, so their casts
    stagger and every next phase's first matmul finds its operand ready
    (no inter-phase bubbles); seed groups slot between ladder phases to
    cover the two cast bursts that do bunch (K^2's c-outer tail).

kernel() takes FULL inputs and returns the FULL output.
"""

import os
import sys
import numpy as np

import concourse.bass as bass
import concourse.tile as tile
import concourse.mybir as mybir
from concourse.bass import ts, ds
from concourse import bass_utils, bacc
from concourse.masks import make_identity

dt = mybir.dt
F32, F32R = dt.float32, dt.float32r

B, D, T_STEPS = 256, 512, 256
NCORES = 8
BC = B // NCORES                # 32 batch rows per core
SB = 16                         # time steps per block
NB = T_STEPS // SB              # 16 blocks (1 seed + 15 rounds)
NR = SB * BC                    # 512 state columns per block
DP = D // 128                   # 4 partition chunks of the feature dim
N_WARMUP = 30                   # identity matmuls before first real MM


def wavefront():
    """(i, j) pairs in anti-diagonal order; i ascending within a group j."""
    for w in range(2 * DP - 1):
        for i in range(max(0, w - DP + 1), min(DP, w + 1)):
            yield i, w - i


def build_nc():
    nc = bacc.Bacc("TRN2", target_bir_lowering=False, debug=False,
                   num_devices=NCORES)
    # all tensor inputs pre-rounded to f32r (e8m11, RNE) on the host
    zt_d = nc.dram_tensor("zt_in", [D, BC], F32R, kind="ExternalInput").ap()
    k_d = nc.dram_tensor("k_in", [D, D], F32R, kind="ExternalInput").ap()
    kt_d = nc.dram_tensor("kt_in", [D, D], F32R, kind="ExternalInput").ap()
    out_d = nc.dram_tensor("out", [D, T_STEPS, BC], F32,
                           kind="ExternalOutput").ap()

    with tile.TileContext(nc) as tc:
        with tc.tile_pool(name="const", bufs=1) as cp, \
             tc.tile_pool(name="pow", bufs=1) as powp, \
             tc.tile_pool(name="state", bufs=3) as stp, \
             tc.tile_pool(name="psum", bufs=1, space="PSUM") as pp, \
             tc.tile_pool(name="psumq", bufs=1, space="PSUM") as pq:
        # PSUM is bank-granular: p0-3 (squarings + rounds) = 4 banks
        #                      + q0-3 (transposes/seed/warmup) = 4 banks

            Kr, KTr = [], []
            for i in range(DP):
                ktr = cp.tile([128, D], F32R, name=f"KTr{i}", tag=f"KTr{i}")
                KTr.append(ktr)
                kr = cp.tile([128, D], F32R, name=f"Kr{i}", tag=f"Kr{i}")
                Kr.append(kr)

            def load_k_chunk(i):
                nc.sync.dma_start(KTr[i][:], kt_d[ts(i, 128), :])
                nc.scalar.dma_start(Kr[i][:], k_d[ts(i, 128), :])

            load_k_chunk(0)

            identf = cp.tile([128, 128], F32, name="identf", tag="identf")
            make_identity(nc, identf[:])
            ident = cp.tile([128, 128], F32R, name="ident", tag="ident")
            nc.vector.tensor_copy(ident[:], identf[:])

            # PE warm-up while the K chunk-0 DMA completion semaphores are
            # still in flight: 4-MM accumulation groups rotating over the
            # q tags (the pattern the steady rounds prove pipelines;
            # single-MM groups serialize on fill/drain and a single long
            # group was measured at isolated-MM cadence too).
            ident2 = cp.tile([128, 128], F32R, name="ident2", tag="ident2")
            nc.scalar.copy(ident2[:], identf[:])
            with nc.named_scope("warmup"):
                for g in range(N_WARMUP // 4):
                    pw = pq.tile([128, 128], F32, name=f"wu{g}",
                                 tag=f"q{g % 4}")
                    for w in range(4):
                        lhs = ident if w % 2 == 0 else ident2
                        nc.tensor.matmul(pw[:], lhs[:], ident[:],
                                         start=(w == 0), stop=(w == 3))

            def square(ATr, Ar, name, c_outer=False):
                """(A @ A) as f32r tiles. ATr: lhsT (A^T); Ar: rhs (A)."""
                pss = [pp.tile([128, D], F32, name=f"sq_{name}_{ib}",
                               tag=f"p{ib}") for ib in range(DP)]
                order = ([(c, ib) for c in range(DP) for ib in range(DP)]
                         if c_outer else
                         [(c, ib) for ib in range(DP) for c in range(DP)])
                for c, ib in order:
                    if c_outer and ib == 0 and c + 1 < DP:
                        load_k_chunk(c + 1)
                    if c_outer and ib == 0 and c > 0:
                        # trivial identity transpose: breaks the coalesced
                        # PE semaphore run so this c-group waits only on
                        # its own K chunks, not on all of them.
                        dum = pq.tile([128, 128], F32R, name=f"dum{name}{c}",
                                      tag=f"q{c}")
                        nc.tensor.transpose(dum[:], ident[:], ident[:])
                    nc.tensor.matmul(pss[ib][:],
                                     ATr[c][:, ts(ib, 128)],
                                     Ar[c][:],
                                     start=(c == 0), stop=(c == DP - 1))
                out_tiles = []
                for ib in range(DP):
                    r = powp.tile([128, D], F32R, name=f"{name}r_{ib}",
                                  tag=f"{name}r_{ib}")
                    if ib % 2 == 0:
                        nc.vector.tensor_copy(r[:], pss[ib][:])
                    else:
                        nc.scalar.copy(r[:], pss[ib][:])
                    out_tiles.append(r)
                return out_tiles

            def transpose_chunk(Ar, outs, c, name):
                """PE-transpose chunk c: outs[c][:, i-slice] = Ar[i][:,c].T"""
                for i in range(DP):
                    ps = pq.tile([128, 128], F32R, name=f"t_{name}_{i}{c}",
                                 tag=f"q{i}")
                    nc.tensor.transpose(ps[:], Ar[i][:, ts(c, 128)],
                                        ident[:])
                    if (i + c) % 2 == 0:
                        nc.vector.tensor_copy(outs[c][:, ts(i, 128)], ps[:])
                    else:
                        nc.scalar.copy(outs[c][:, ts(i, 128)], ps[:])

            def fused_transpose_square(Ar, tname, sqname):
                """Transpose A chunk-by-chunk, interleaved with the c-groups
                of the squaring A@A that consumes the transposed chunks —
                keeps the PE array duty high enough that the HAM never
                re-throttles, and every matmul finds its lhsT cast done."""
                ATr = [powp.tile([128, D], F32R, name=f"{tname}_{c}",
                                 tag=f"{tname}_{c}") for c in range(DP)]
                pss = [pp.tile([128, D], F32, name=f"sq_{sqname}_{ib}",
                               tag=f"p{ib}") for ib in range(DP)]
                transpose_chunk(Ar, ATr, 0, tname)
                transpose_chunk(Ar, ATr, 1, tname)
                for c in range(DP):
                    if c + 2 < DP:
                        transpose_chunk(Ar, ATr, c + 2, tname)
                    for ib in range(DP):
                        nc.tensor.matmul(pss[ib][:],
                                         ATr[c][:, ts(ib, 128)],
                                         Ar[c][:],
                                         start=(c == 0), stop=(c == DP - 1))
                out_tiles = []
                for ib in range(DP):
                    r = powp.tile([128, D], F32R, name=f"{sqname}r_{ib}",
                                  tag=f"{sqname}r_{ib}")
                    if ib % 2 == 0:
                        nc.vector.tensor_copy(r[:], pss[ib][:])
                    else:
                        nc.scalar.copy(r[:], pss[ib][:])
                    out_tiles.append(r)
                return out_tiles

            # seed block: X[ib][:, s*BC:(s+1)*BC] = (z_{s+1})^T chunk,
            # s = 0..15
            X = [stp.tile([128, NR], F32R, name=f"x0_{ib}",
                          tag=f"x{ib}") for ib in range(DP)]

            def seed_group(lhsT, rhs_of, n, dst_lo, name):
                pss = [pq.tile([128, n], F32, name=f"{name}_{jb}",
                               tag=f"q{jb}") for jb in range(DP)]
                done = [0] * DP
                for i, jb in wavefront():
                    nc.tensor.matmul(pss[jb][:],
                                     lhsT[i][:, ts(jb, 128)],
                                     rhs_of(i),
                                     start=(i == 0), stop=(i == DP - 1))
                    done[jb] += 1
                    if done[jb] == DP:
                        dst = X[jb][:, dst_lo * BC:dst_lo * BC + n]
                        if jb % 2 == 0:
                            nc.vector.tensor_copy(dst, pss[jb][:])
                        else:
                            nc.scalar.copy(dst, pss[jb][:])

            with nc.named_scope("ladder"):
                K2r = square(KTr, Kr, "K2", c_outer=True)
            z0t = [cp.tile([128, BC], F32R, name=f"z0t{i}", tag=f"z0t{i}")
                   for i in range(DP)]
            for i in range(DP):
                nc.sync.dma_start(z0t[i][:], zt_d[ts(i, 128), :])

            with nc.named_scope("seed"):
                seed_group(Kr, lambda i: z0t[i][:], BC, 0, "s0")
            with nc.named_scope("ladder"):
                K4r = fused_transpose_square(K2r, "K2T", "K4")
            with nc.named_scope("seed"):
                seed_group(Kr, lambda i: X[i][:, 0:BC], BC, 1, "s1")
            with nc.named_scope("ladder"):
                K8r = fused_transpose_square(K4r, "K4T", "K8")
            with nc.named_scope("seed"):
                seed_group(K2r, lambda i: X[i][:, 0:2 * BC], 2 * BC, 2, "s2")
            with nc.named_scope("ladder"):
                K16r = fused_transpose_square(K8r, "K8T", "K16")
            with nc.named_scope("seed"):
                seed_group(K4r, lambda i: X[i][:, 0:4 * BC], 4 * BC, 4, "s3")
                seed_group(K8r, lambda i: X[i][:, 0:8 * BC], 8 * BC, 8, "s4")
            # seed block -> output rows 0..15
            for ib in range(DP):
                dma_eng = nc.sync if ib % 2 == 0 else nc.scalar
                dma_eng.dma_start(
                    out_d[ts(ib, 128), ds(0, SB), :],
                    X[ib][:].bitcast(F32))

            # ---------------- phase B: K^16 rounds ----------------
            with nc.named_scope("rounds"):
                for r in range(1, NB):
                    pss = [pp.tile([128, NR], F32, name=f"rd{r}_{jb}",
                                   tag=f"p{jb}") for jb in range(DP)]
                    done = [0] * DP
                    nxt = [None] * DP
                    for i, jb in wavefront():
                        nc.tensor.matmul(pss[jb][:],
                                         K16r[i][:, ts(jb, 128)],
                                         X[i][:],
                                         start=(i == 0), stop=(i == DP - 1))
                        done[jb] += 1
                        if done[jb] == DP:
                            o = stp.tile([128, NR], F32R,
                                         name=f"x{r}_{jb}", tag=f"x{jb}")
                            nc.vector.tensor_copy(o[:], pss[jb][:])
                            dma_eng = nc.sync if jb % 2 == 0 else nc.scalar
                            dma_eng.dma_start(
                                out_d[ts(jb, 128), ds(SB * r, SB), :],
                                o[:].bitcast(F32))
                            nxt[jb] = o
                    X = nxt

    nc.compile()
    return nc


def _round_f32r(x):
    """RNE round fp32 -> f32r (e8m11): bit-exact match of the HW/DVE cast."""
    b = x.view(np.uint32).astype(np.uint64)
    keep = b >> 12
    rem = b & 0xFFF
    rup = (rem > 0x800) | ((rem == 0x800) & ((keep & 1) == 1))
    return ((keep + rup) << 12).astype(np.uint32).view(np.float32).copy()


_CACHE = {}


def kernel(z0, K, T):
    z0 = np.asarray(z0, dtype=np.float32)
    K = np.asarray(K, dtype=np.float32)
    T = int(T)
    assert z0.shape == (B, D) and K.shape == (D, D) and T == T_STEPS

    if "nc" not in _CACHE:
        _CACHE["nc"] = build_nc()
    nc = _CACHE["nc"]

    Kr = _round_f32r(np.ascontiguousarray(K))
    zt = _round_f32r(np.ascontiguousarray(z0.T))      # [D, B]
    kt = np.ascontiguousarray(Kr.T)                   # [D, D] (round then T)
    in_maps = []
    for m in range(NCORES):
        in_maps.append({
            "zt_in": np.ascontiguousarray(zt[:, m * BC:(m + 1) * BC]),
            "k_in": Kr, "kt_in": kt})

    trace = bool(os.environ.get("KOOPMAN_TRACE"))
    if trace:
        _install_ntff_hook()
    res = bass_utils.run_bass_kernel_spmd(
        nc, in_maps, core_ids=list(range(NCORES)),
        trace=trace, trace_cores=[0] if trace else None)
    if trace:
        _CACHE["last_result"] = res

    # assemble: per-core out [D, T, BC] -> full [B, T, D]
    full = np.empty((B, T_STEPS, D), dtype=np.float32)
    for m in range(NCORES):
        blk = res.results[m]["out"]               # [D, T, BC]
        full[m * BC:(m + 1) * BC, :, :] = blk.transpose(2, 1, 0)
    return full


def _install_ntff_hook():
    """Dev-only: register the axon NTFF profiling hook (absent from this
    image's antenv) so trace=True works."""
    import types
    if "antenv.axon_hooks" in sys.modules:
        return
    try:
        from trn_agent_boot.trn_boot import _ntff_profile_via_ctypes
        hook = _ntff_profile_via_ctypes("/opt/axon/libaxon_pjrt.so")
    except Exception:
        return
    mod = types.ModuleType("antenv.axon_hooks")
    mod.get_axon_ntff_profile_hook = lambda: hook
    mod.set_axon_ntff_profile_hook = lambda h: None
    sys.modules["antenv.axon_hooks"] = mod


# revision 32
# speedup vs baseline: 1.0851x; 1.0600x over previous
"""TRN2 Bass kernel for the discrete dense Koopman operator rollout.

    z_{t+1} = z_t @ K ;  output[b, t, d] = (z0 @ K^{t+1})[b, d],  t = 0..255

Strategy (batch sharding, SPMD across 8 NeuronCores):
  - core m owns batch rows 32m .. 32m+31 and computes ALL 256 time steps
    for them (the scan is embarrassingly parallel over batch).
  - on-device setup per core (identical instruction stream):
      * squaring ladder K -> K^2 -> K^4 -> K^8 -> K^16 (4 squarings;
        host supplies K^T to seed the first squaring; intermediate
        transposes via PE transpose-mode)
      * seed rows z_1..z_16 by state doubling:
          z_1 = z_0 K ; z_2 = z_1 K ; [z_3 z_4] = [z_1 z_2] K^2 ;
          [z_5..z_8] = [z_1..z_4] K^4 ; [z_9..z_16] = [z_1..z_8] K^8
      * 15 rounds: block_{r+1} = block_r @ K^16, where a block is 16
        time steps x 32 batch = a [512, 512] feature-major tile group ->
        each round is 16 accumulating [128x128]@[128,512] matmuls (the
        moving stream fully hides the f32r weight load, and casts/DMAs
        have ~3 us of slack inside a 3.6 us round -> no just-in-time
        stalls on the PE).
  - matmuls run as float32r (e8m11, RNE; 1 cycle/row).  Inputs are
    pre-rounded on the host (bit-exact same RNE); PSUM accumulation is
    exact fp32; the rounded state is DMA'd out directly as fp32 output.
  - state kept feature-major (ZT = z^T, [D, b]) so K blocks are the
    stationary operand and no per-step transposes are needed.
  - output DRAM layout is feature-major [D, T, BC] so each output DMA
    writes 2 KiB contiguous per partition (time-major would shatter
    DMAs into 128 B segments at ~4x the cost); the host transposes.
  - startup: K^T/K chunk i ride the two HWDGE queues in lockstep and the
    first squaring consumes them in arrival order; z0 queues behind the
    K^T loads.  A single long accumulation group of identity matmuls
    warms the PE (HAM un-throttle) while the first K chunks' DMA
    completion semaphores are still in flight.
  - squarings after the first run output-chunk-outer, so their casts
    stagger and every next phase's first matmul finds its operand ready
    (no inter-phase bubbles); seed groups slot between ladder phases to
    cover the two cast bursts that do bunch (K^2's c-outer tail).

kernel() takes FULL inputs and returns the FULL output.
"""

import os
import sys
import numpy as np

import concourse.bass as bass
import concourse.tile as tile
import concourse.mybir as mybir
from concourse.bass import ts, ds
from concourse import bass_utils, bacc
from concourse.masks import make_identity

dt = mybir.dt
F32, F32R = dt.float32, dt.float32r

B, D, T_STEPS = 256, 512, 256
NCORES = 8
BC = B // NCORES                # 32 batch rows per core
SB = 16                         # time steps per block
NB = T_STEPS // SB              # 16 blocks (1 seed + 15 rounds)
NR = SB * BC                    # 512 state columns per block
DP = D // 128                   # 4 partition chunks of the feature dim
N_WARMUP = 8                    # N=512 zero-matmuls before first real MM


def wavefront():
    """(i, j) pairs in anti-diagonal order; i ascending within a group j."""
    for w in range(2 * DP - 1):
        for i in range(max(0, w - DP + 1), min(DP, w + 1)):
            yield i, w - i


def build_nc():
    nc = bacc.Bacc("TRN2", target_bir_lowering=False, debug=False,
                   num_devices=NCORES)
    # all tensor inputs pre-rounded to f32r (e8m11, RNE) on the host
    zt_d = nc.dram_tensor("zt_in", [D, BC], F32R, kind="ExternalInput").ap()
    k_d = nc.dram_tensor("k_in", [D, D], F32R, kind="ExternalInput").ap()
    kt_d = nc.dram_tensor("kt_in", [D, D], F32R, kind="ExternalInput").ap()
    out_d = nc.dram_tensor("out", [D, T_STEPS, BC], F32,
                           kind="ExternalOutput").ap()

    with tile.TileContext(nc) as tc:
        with tc.tile_pool(name="const", bufs=1) as cp, \
             tc.tile_pool(name="pow", bufs=1) as powp, \
             tc.tile_pool(name="state", bufs=3) as stp, \
             tc.tile_pool(name="psum", bufs=1, space="PSUM") as pp, \
             tc.tile_pool(name="psumq", bufs=1, space="PSUM") as pq:
        # PSUM is bank-granular: p0-3 (squarings + rounds) = 4 banks
        #                      + q0-3 (transposes/seed/warmup) = 4 banks

            Kr, KTr = [], []
            for i in range(DP):
                ktr = cp.tile([128, D], F32R, name=f"KTr{i}", tag=f"KTr{i}")
                KTr.append(ktr)
                kr = cp.tile([128, D], F32R, name=f"Kr{i}", tag=f"Kr{i}")
                Kr.append(kr)

            def load_k_chunk(i):
                nc.sync.dma_start(KTr[i][:], kt_d[ts(i, 128), :])
                nc.scalar.dma_start(Kr[i][:], k_d[ts(i, 128), :])

            load_k_chunk(0)

            # warm-up operands: zero tiles via DVE memset — no gpsimd
            # dependency, so the PE can start ~1 us earlier than waiting
            # for the identity build.
            wu_lhs = cp.tile([128, 128], F32, name="wu_lhs", tag="wu_lhs")
            nc.vector.memset(wu_lhs[:], 0.0)
            wu_rhs = cp.tile([128, D], F32, name="wu_rhs", tag="wu_rhs")
            nc.vector.memset(wu_rhs[:], 0.0)
            # N=512: 80% array duty even though back-to-back N=128 matmuls
            # cannot overlap fill with drain (measured: any N=128 stream
            # runs at isolated-MM cadence) — dense enough to flip the HAM.
            # On tag p0 so the first squaring queues strictly behind it.
            with nc.named_scope("warmup"):
                pw = pp.tile([128, D], F32, name="wu", tag="p0")
                for w in range(N_WARMUP):
                    nc.tensor.matmul(pw[:], wu_lhs[:].bitcast(F32R),
                                     wu_rhs[:].bitcast(F32R),
                                     start=(w == 0),
                                     stop=(w == N_WARMUP - 1))

            identf = cp.tile([128, 128], F32, name="identf", tag="identf")
            make_identity(nc, identf[:])
            ident = cp.tile([128, 128], F32R, name="ident", tag="ident")
            nc.vector.tensor_copy(ident[:], identf[:])

            def square(ATr, Ar, name, c_outer=False):
                """(A @ A) as f32r tiles. ATr: lhsT (A^T); Ar: rhs (A)."""
                pss = [pp.tile([128, D], F32, name=f"sq_{name}_{ib}",
                               tag=f"p{ib}") for ib in range(DP)]
                order = ([(c, ib) for c in range(DP) for ib in range(DP)]
                         if c_outer else
                         [(c, ib) for ib in range(DP) for c in range(DP)])
                for c, ib in order:
                    if c_outer and ib == 0 and c + 1 < DP:
                        load_k_chunk(c + 1)
                    if c_outer and ib == 0 and c > 0:
                        # trivial identity transpose: breaks the coalesced
                        # PE semaphore run so this c-group waits only on
                        # its own K chunks, not on all of them.
                        dum = pq.tile([128, 128], F32R, name=f"dum{name}{c}",
                                      tag=f"q{c}")
                        nc.tensor.transpose(dum[:], ident[:], ident[:])
                    nc.tensor.matmul(pss[ib][:],
                                     ATr[c][:, ts(ib, 128)],
                                     Ar[c][:],
                                     start=(c == 0), stop=(c == DP - 1))
                out_tiles = []
                for ib in range(DP):
                    r = powp.tile([128, D], F32R, name=f"{name}r_{ib}",
                                  tag=f"{name}r_{ib}")
                    if ib % 2 == 0:
                        nc.vector.tensor_copy(r[:], pss[ib][:])
                    else:
                        nc.scalar.copy(r[:], pss[ib][:])
                    out_tiles.append(r)
                return out_tiles

            def transpose_chunk(Ar, outs, c, name):
                """PE-transpose chunk c: outs[c][:, i-slice] = Ar[i][:,c].T"""
                for i in range(DP):
                    ps = pq.tile([128, 128], F32R, name=f"t_{name}_{i}{c}",
                                 tag=f"q{i}")
                    nc.tensor.transpose(ps[:], Ar[i][:, ts(c, 128)],
                                        ident[:])
                    if (i + c) % 2 == 0:
                        nc.vector.tensor_copy(outs[c][:, ts(i, 128)], ps[:])
                    else:
                        nc.scalar.copy(outs[c][:, ts(i, 128)], ps[:])

            def fused_transpose_square(Ar, tname, sqname):
                """Transpose A chunk-by-chunk, interleaved with the c-groups
                of the squaring A@A that consumes the transposed chunks —
                keeps the PE array duty high enough that the HAM never
                re-throttles, and every matmul finds its lhsT cast done."""
                ATr = [powp.tile([128, D], F32R, name=f"{tname}_{c}",
                                 tag=f"{tname}_{c}") for c in range(DP)]
                pss = [pp.tile([128, D], F32, name=f"sq_{sqname}_{ib}",
                               tag=f"p{ib}") for ib in range(DP)]
                transpose_chunk(Ar, ATr, 0, tname)
                transpose_chunk(Ar, ATr, 1, tname)
                for c in range(DP):
                    if c + 2 < DP:
                        transpose_chunk(Ar, ATr, c + 2, tname)
                    for ib in range(DP):
                        nc.tensor.matmul(pss[ib][:],
                                         ATr[c][:, ts(ib, 128)],
                                         Ar[c][:],
                                         start=(c == 0), stop=(c == DP - 1))
                out_tiles = []
                for ib in range(DP):
                    r = powp.tile([128, D], F32R, name=f"{sqname}r_{ib}",
                                  tag=f"{sqname}r_{ib}")
                    if ib % 2 == 0:
                        nc.vector.tensor_copy(r[:], pss[ib][:])
                    else:
                        nc.scalar.copy(r[:], pss[ib][:])
                    out_tiles.append(r)
                return out_tiles

            # seed block: X[ib][:, s*BC:(s+1)*BC] = (z_{s+1})^T chunk,
            # s = 0..15
            X = [stp.tile([128, NR], F32R, name=f"x0_{ib}",
                          tag=f"x{ib}") for ib in range(DP)]

            def seed_group(lhsT, rhs_of, n, dst_lo, name):
                pss = [pq.tile([128, n], F32, name=f"{name}_{jb}",
                               tag=f"q{jb}") for jb in range(DP)]
                done = [0] * DP
                for i, jb in wavefront():
                    nc.tensor.matmul(pss[jb][:],
                                     lhsT[i][:, ts(jb, 128)],
                                     rhs_of(i),
                                     start=(i == 0), stop=(i == DP - 1))
                    done[jb] += 1
                    if done[jb] == DP:
                        dst = X[jb][:, dst_lo * BC:dst_lo * BC + n]
                        if jb % 2 == 0:
                            nc.vector.tensor_copy(dst, pss[jb][:])
                        else:
                            nc.scalar.copy(dst, pss[jb][:])

            with nc.named_scope("ladder"):
                K2r = square(KTr, Kr, "K2", c_outer=True)
            z0t = [cp.tile([128, BC], F32R, name=f"z0t{i}", tag=f"z0t{i}")
                   for i in range(DP)]
            for i in range(DP):
                nc.sync.dma_start(z0t[i][:], zt_d[ts(i, 128), :])

            with nc.named_scope("seed"):
                seed_group(Kr, lambda i: z0t[i][:], BC, 0, "s0")
            with nc.named_scope("ladder"):
                K4r = fused_transpose_square(K2r, "K2T", "K4")
            with nc.named_scope("seed"):
                seed_group(Kr, lambda i: X[i][:, 0:BC], BC, 1, "s1")
            with nc.named_scope("ladder"):
                K8r = fused_transpose_square(K4r, "K4T", "K8")
            with nc.named_scope("seed"):
                seed_group(K2r, lambda i: X[i][:, 0:2 * BC], 2 * BC, 2, "s2")
            with nc.named_scope("ladder"):
                K16r = fused_transpose_square(K8r, "K8T", "K16")
            with nc.named_scope("seed"):
                seed_group(K4r, lambda i: X[i][:, 0:4 * BC], 4 * BC, 4, "s3")
                seed_group(K8r, lambda i: X[i][:, 0:8 * BC], 8 * BC, 8, "s4")
            # seed block -> output rows 0..15
            for ib in range(DP):
                dma_eng = nc.sync if ib % 2 == 0 else nc.scalar
                dma_eng.dma_start(
                    out_d[ts(ib, 128), ds(0, SB), :],
                    X[ib][:].bitcast(F32))

            # ---------------- phase B: K^16 rounds ----------------
            with nc.named_scope("rounds"):
                for r in range(1, NB):
                    # alternate between the two 4-bank PSUM pools so the
                    # bank-reuse WAR is two rounds deep, not one
                    rp = pp if r % 2 == 1 else pq
                    tg = "p" if r % 2 == 1 else "q"
                    pss = [rp.tile([128, NR], F32, name=f"rd{r}_{jb}",
                                   tag=f"{tg}{jb}") for jb in range(DP)]
                    done = [0] * DP
                    nxt = [None] * DP
                    for i, jb in wavefront():
                        nc.tensor.matmul(pss[jb][:],
                                         K16r[i][:, ts(jb, 128)],
                                         X[i][:],
                                         start=(i == 0), stop=(i == DP - 1))
                        done[jb] += 1
                        if done[jb] == DP:
                            o = stp.tile([128, NR], F32R,
                                         name=f"x{r}_{jb}", tag=f"x{jb}")
                            nc.vector.tensor_copy(o[:], pss[jb][:])
                            dma_eng = nc.sync if jb % 2 == 0 else nc.scalar
                            dma_eng.dma_start(
                                out_d[ts(jb, 128), ds(SB * r, SB), :],
                                o[:].bitcast(F32))
                            nxt[jb] = o
                    X = nxt

    nc.compile()
    return nc


def _round_f32r(x):
    """RNE round fp32 -> f32r (e8m11): bit-exact match of the HW/DVE cast."""
    b = x.view(np.uint32).astype(np.uint64)
    keep = b >> 12
    rem = b & 0xFFF
    rup = (rem > 0x800) | ((rem == 0x800) & ((keep & 1) == 1))
    return ((keep + rup) << 12).astype(np.uint32).view(np.float32).copy()


_CACHE = {}


def kernel(z0, K, T):
    z0 = np.asarray(z0, dtype=np.float32)
    K = np.asarray(K, dtype=np.float32)
    T = int(T)
    assert z0.shape == (B, D) and K.shape == (D, D) and T == T_STEPS

    if "nc" not in _CACHE:
        _CACHE["nc"] = build_nc()
    nc = _CACHE["nc"]

    Kr = _round_f32r(np.ascontiguousarray(K))
    zt = _round_f32r(np.ascontiguousarray(z0.T))      # [D, B]
    kt = np.ascontiguousarray(Kr.T)                   # [D, D] (round then T)
    in_maps = []
    for m in range(NCORES):
        in_maps.append({
            "zt_in": np.ascontiguousarray(zt[:, m * BC:(m + 1) * BC]),
            "k_in": Kr, "kt_in": kt})

    trace = bool(os.environ.get("KOOPMAN_TRACE"))
    if trace:
        _install_ntff_hook()
    res = bass_utils.run_bass_kernel_spmd(
        nc, in_maps, core_ids=list(range(NCORES)),
        trace=trace, trace_cores=[0] if trace else None)
    if trace:
        _CACHE["last_result"] = res

    # assemble: per-core out [D, T, BC] -> full [B, T, D]
    full = np.empty((B, T_STEPS, D), dtype=np.float32)
    for m in range(NCORES):
        blk = res.results[m]["out"]               # [D, T, BC]
        full[m * BC:(m + 1) * BC, :, :] = blk.transpose(2, 1, 0)
    return full


def _install_ntff_hook():
    """Dev-only: register the axon NTFF profiling hook (absent from this
    image's antenv) so trace=True works."""
    import types
    if "antenv.axon_hooks" in sys.modules:
        return
    try:
        from trn_agent_boot.trn_boot import _ntff_profile_via_ctypes
        hook = _ntff_profile_via_ctypes("/opt/axon/libaxon_pjrt.so")
    except Exception:
        return
    mod = types.ModuleType("antenv.axon_hooks")
    mod.get_axon_ntff_profile_hook = lambda: hook
    mod.set_axon_ntff_profile_hook = lambda h: None
    sys.modules["antenv.axon_hooks"] = mod
